# revision 19
# baseline (speedup 1.0000x reference)
"""BitLinear (ternary weight) inference kernel for Trainium2, 8-core SPMD.

Full-input contract: kernel(**inputs) takes the complete tensors and returns
the complete output. The batch dim (B=8) is sharded 1:1 onto the 8
NeuronCores; each core computes y[b] = x[b] @ (w_q * 2^s_exp)^T + bias as a
2048^3 matmul.

All-fp8 DoubleRow scheme. The accuracy gate is max|err|/absmax, and both
error and signal in column o scale with 2^s_exp[o]:
  - Output columns are permuted by s_exp descending. ALL columns run
    x8 (e4m3) x w8 (e4m3) DoubleRow matmuls (K=256/instr, 1 col/cycle =
    2x bf16 rate, the fp8 ceiling on trn2 HW).
  - The top RES=256 columns (covering all s_exp=0) get a second
    DoubleRow pass with xlo8 = e4m3(x - x8) into the same PSUM bank
    (two-term fp8, ~2^-8 relative x error). Remaining columns carry
    error scaled by 2^s <= 1/2. Measured: rel 1.15e-2 vs 2e-2 gate.
  - Weights +-2^s / 0 are EXACT in fp8e4m3.
  - PE streaming floor: 16 tiles x 8 kp x (2048 + 256) cols = 122.9us.

Schedule (driven by perfetto traces of v1/v2):
  - Framework preamble runs to ~7.2us (host event + engine init; fixed).
    Warm-up DR matmuls on an *unwritten* tile start right at preamble
    end (no memset dependency) and ride the HAM clock ramp.
  - Work = (tile, column-chunk) groups of 8 (16 with residual) DR
    matmuls into one PSUM bank; epilogue on vector adds bias -> fp16
    slice; one contiguous per-chunk store per group.
  - Chunks c1a/c1b load on sync, c2a/c2b/c3/c0 on scalar, so two
    256-col sweeps are ready by ~12us from both rings; x8 tiles + bias
    ride gpsimd SWDGE (first tiles individually for fast start, rest in
    bulk blocks to amortize the ~0.7us/issue engine cost); xlo bulks
    follow on sync (needed only by the final c0 sweep).
  - A build-time greedy orders groups against a calibrated arrival
    model (ring rates, issue costs, HAM ramp) for zero PE gaps.
  - y is stored per-chunk ([T, W] tensors) so each [128, W] store is
    one fully contiguous 64-128KB burst; host reassembles.
"""
import os

import ml_dtypes
import numpy as np

B, T, IN, OUT = 8, 2048, 2048, 2048
P = 128
NCORES = 8
KP = IN // (2 * P)  # 8 k-pairs (DoubleRow K=256 per instruction)
TT = T // P         # 16 row tiles
RES = 256           # columns (after perm) that get the xlo residual pass
NWARM = 7

# Column chunks: (name, col_lo, col_hi, residual). Ring assignment below.
CHUNKS = [
    ("c1a", 512, 768, False),
    ("c1b", 768, 1024, False),
    ("c2a", 1024, 1280, False),
    ("c2b", 1280, 1536, False),
    ("c3", 1536, 2048, False),
    ("c0", 0, 512, True),
]
SYNC_W = ("c1a", "c1b")          # weight chunks on the sync ring
SCALAR_W = ("c2a", "c2b", "c3", "c0")

# x8 tile DMA blocks on gpsimd: (start, end) tile ranges; bias rides
# after block 2 (see _build). First tile split in halves.
X8_BLOCKS = [(0, 1), (1, 2), (2, 3), (3, 4), (4, 8), (8, 12), (12, 16)]

# Arrival model (us, MB/us) calibrated on v2/v3 perfetto traces.
# Early-kernel ring rates (clock-ramp affected): sync ~180-200GB/s,
# scalar ~95-100, gpsimd SWDGE ~50-60.
T_PRE = 7.6      # engines free after framework preamble
T_ISSUE = 0.7    # per dma_start instruction on the issuing engine
BW_SY = 0.18
BW_SC = 0.095
BW_GP = 0.10   # slow (~0.05) pre-ramp, faster later; xlo deadline is loose
RAMP_END = 14.0  # PE at ~half clock before this (HAM ramp)
PE_NS_PER_COL = 1 / 2.4e3

last_exec_time_ns = None
_CACHE = {}


def _install_prof_shim():
    """Make antenv.axon_hooks importable so trace=True works under axon."""
    import sys
    import types

    if "antenv.axon_hooks" in sys.modules:
        return
    try:
        from trn_agent_boot.trn_boot import _ntff_profile_via_ctypes
    except ImportError:
        return
    hook = _ntff_profile_via_ctypes("/opt/axon/libaxon_pjrt.so")
    mod = types.ModuleType("antenv.axon_hooks")
    mod.get_axon_ntff_profile_hook = lambda: hook
    mod.set_axon_ntff_profile_hook = lambda h: None
    sys.modules["antenv.axon_hooks"] = mod


def _ring(sizes_mb, bw):
    """Completion times for sequential ring transfers with issue costs."""
    out = []
    t = 0.0
    for i, s in enumerate(sizes_mb):
        t = max(t, T_PRE + T_ISSUE * (i + 1)) + s / bw
        out.append(t)
    return out


def _arrivals():
    # sync: c1a, x8[0] halves, x8[1], c1b, x8[2], x8[3], three 4-tile bulks
    sy = _ring([0.25, 0.125, 0.125, 0.25, 0.25, 0.25, 0.25, 1.0, 1.0, 1.0],
               BW_SY)
    arr_x8 = {0: sy[2], 1: sy[3], 2: sy[5], 3: sy[6]}
    for tt in range(4, 8):
        arr_x8[tt] = sy[7]
    for tt in range(8, 12):
        arr_x8[tt] = sy[8]
    for tt in range(12, 16):
        arr_x8[tt] = sy[9]

    # scalar: bias[512:1024], c2a, c2b, c3, c0
    sc = _ring([0.25, 0.25, 0.25, 0.5, 0.5], BW_SC)
    arr_w = {"c1a": sy[0], "c1b": sy[4],
             "c2a": sc[1], "c2b": sc[2], "c3": sc[3], "c0": sc[4]}

    # gpsimd: bias slices, xlo in two bulks (needed only by c0 sweep)
    gp = _ring([0.25, 0.25, 0.25, 2.0, 2.0], BW_GP)
    arr_xlo = {tt: gp[3] if tt < 8 else gp[4] for tt in range(TT)}
    arr_bias = sc[0]
    return arr_x8, arr_w, arr_xlo, arr_bias


def _schedule():
    """Greedy (chunk, tile) order from the arrival model."""
    arr_x8, arr_w, arr_xlo, _ = _arrivals()
    groups = []
    for ci, (name, lo, hi, res) in enumerate(CHUNKS):
        for tt in range(TT):
            ready = max(arr_w[name], arr_x8[tt])
            if res:
                ready = max(ready, arr_xlo[tt])
            cols = (hi - lo) + (RES if res else 0)
            groups.append([ready, ci, tt, cols])

    order = []
    t = T_PRE + 0.45 * NWARM  # warmups at ramp clock
    pend = groups
    while pend:
        ready = [g for g in pend if g[0] <= t]
        if not ready:
            t = min(g[0] for g in pend)
            ready = [g for g in pend if g[0] <= t]
        g = min(ready, key=lambda x: (x[1], x[2]))
        pend.remove(g)
        order.append((g[1], g[2]))
        dur = g[3] * 8 * PE_NS_PER_COL
        if t < RAMP_END:
            dur *= 2
        t += dur
    return order, t


def _build(res):
    import concourse.bacc as bacc
    import concourse.mybir as mybir
    from concourse.tile import TileContext

    DR = mybir.MatmulPerfMode.DoubleRow

    nc = bacc.Bacc()
    # t-major fp8 x tiles: x8lo[tt, p, kp, i, m] = x[tt*128+m, (2kp+i)*128+p]
    # for the first 4 tiles (tile-major, one DMA each for fast start);
    # the remaining 12 tiles are partition-major (x8hi) so 4-tile blocks
    # load as single contiguous DMAs. xlo is partition-major throughout.
    x8lo = nc.dram_tensor("x8lo", (4, P, KP, 2, P), mybir.dt.float8e4,
                          kind="ExternalInput")
    x8hi = nc.dram_tensor("x8hi", (P, TT - 4, KP, 2, P), mybir.dt.float8e4,
                          kind="ExternalInput")
    xlo = nc.dram_tensor("xlo", (P, TT, KP, 2, P), mybir.dt.float8e4,
                         kind="ExternalInput")
    wd = {}
    yd = {}
    for name, lo, hi, _ in CHUNKS:
        wd[name] = nc.dram_tensor(f"w_{name}", (P, KP, 2, hi - lo),
                                  mybir.dt.float8e4, kind="ExternalInput")
        yd[name] = nc.dram_tensor(f"y_{name}", (T, hi - lo),
                                  mybir.dt.float16, kind="ExternalOutput")
    bias = nc.dram_tensor("bias", (P, OUT), mybir.dt.float16,
                          kind="ExternalInput")

    order, _ = _schedule()

    with TileContext(nc) as tc:
        with tc.tile_pool(name="xp", bufs=1) as xp, \
             tc.tile_pool(name="wp", bufs=1) as wp, \
             tc.tile_pool(name="bp", bufs=1) as bp, \
             tc.tile_pool(name="opa", bufs=32) as opa, \
             tc.tile_pool(name="opb", bufs=20) as opb, \
             tc.tile_pool(name="pp", bufs=8, space="PSUM") as pp:

            # HAM pre-warm: small memset on gpsimd (its first instruction,
            # ~0.3us), then DR matmuls ride the clock ramp while loads land.
            warm_sb = bp.tile([P, 2, 256], mybir.dt.float8e4, tag="warm")
            nc.gpsimd.memset(warm_sb, 0.0)
            warm_ps = pp.tile([P, 256], mybir.dt.float32, tag="ps",
                              name="warmps")
            for i in range(NWARM):
                nc.tensor.matmul(warm_ps, warm_sb[:, :, 0:128], warm_sb,
                                 start=(i == 0), stop=(i == NWARM - 1),
                                 perf_mode=DR)

            # --- input loads ---
            x8_sb = [xp.tile([P, KP, 2, P], mybir.dt.float8e4,
                             tag=f"x8_{tt}", name=f"x8_{tt}")
                     for tt in range(4)]
            x8hi_sb = xp.tile([P, TT - 4, KP, 2, P], mybir.dt.float8e4,
                              tag="x8hi")
            xlo_sb = xp.tile([P, TT, KP, 2, P], mybir.dt.float8e4,
                             tag="xlo")
            bias_sb = bp.tile([P, OUT], mybir.dt.float16, tag="bias")

            w_sb = {}
            for name, lo, hi, _ in CHUNKS:
                w_sb[name] = wp.tile([P, KP, 2, hi - lo], mybir.dt.float8e4,
                                     tag=f"w_{name}", name=f"w_{name}")

            # sync HWDGE (fastest early): c1a, x8[0] halves, x8[1], c1b,
            # x8[2], x8[3], then the 12 high tiles in three 4-tile blocks
            nc.sync.dma_start(w_sb["c1a"], wd["c1a"][:, :, :, :])
            for q in range(0, KP, 4):
                nc.sync.dma_start(x8_sb[0][:, q:q + 4], x8lo[0, :, q:q + 4])
            nc.sync.dma_start(x8_sb[1], x8lo[1])
            nc.sync.dma_start(w_sb["c1b"], wd["c1b"][:, :, :, :])
            nc.sync.dma_start(x8_sb[2], x8lo[2])
            nc.sync.dma_start(x8_sb[3], x8lo[3])
            for j in range(0, TT - 4, 4):
                nc.sync.dma_start(x8hi_sb[:, j:j + 4], x8hi[:, j:j + 4])

            # scalar HWDGE: first bias slice, then remaining weight chunks
            nc.scalar.dma_start(bias_sb[:, 512:1024], bias[:, 512:1024])
            for name in SCALAR_W:
                nc.scalar.dma_start(w_sb[name], wd[name][:, :, :, :])

            # gpsimd SWDGE (slowest early): bias tail slices, xlo bulks
            nc.gpsimd.dma_start(bias_sb[:, 1024:1536], bias[:, 1024:1536])
            nc.gpsimd.dma_start(bias_sb[:, 1536:2048], bias[:, 1536:2048])
            nc.gpsimd.dma_start(bias_sb[:, 0:512], bias[:, 0:512])
            for j in range(0, TT, 8):
                nc.gpsimd.dma_start(xlo_sb[:, j:j + 8], xlo[:, j:j + 8])

            def x8_ap(tt, kp):
                if tt < 4:
                    return x8_sb[tt][:, kp, :, :]
                return x8hi_sb[:, tt - 4, kp, :, :]

            # --- compute groups ---
            def group(gi, ci, tt):
                name, lo, hi, has_res = CHUNKS[ci]
                w = hi - lo
                wt = w_sb[name]
                ps = pp.tile([P, w], mybir.dt.float32, tag="ps",
                             name=f"ps_{name}_{tt}")
                for kp in range(KP):
                    nc.tensor.matmul(ps, x8_ap(tt, kp),
                                     wt[:, kp, :, :],
                                     start=(kp == 0),
                                     stop=(kp == KP - 1 and not has_res),
                                     perf_mode=DR)
                if has_res:
                    for kp in range(KP):
                        nc.tensor.matmul(ps[:, :res],
                                         xlo_sb[:, tt, kp, :, :],
                                         wt[:, kp, :, 0:res],
                                         start=False, stop=(kp == KP - 1),
                                         perf_mode=DR)
                pool = opa if w == 256 else opb
                ot = pool.tile([P, w], mybir.dt.float16, tag="out",
                               name=f"ot_{name}_{tt}")
                nc.vector.tensor_add(ot, ps, bias_sb[:, lo:hi])
                eng = nc.scalar if gi % 2 == 0 else nc.sync
                eng.dma_start(yd[name][tt * P:(tt + 1) * P, :], ot)

            for gi, (ci, tt) in enumerate(order):
                group(gi, ci, tt)

    nc.compile()
    return nc


def kernel(x, w_q, s_exp, bias):
    global last_exec_time_ns
    from concourse.bass_utils import run_bass_kernel_spmd

    f8 = ml_dtypes.float8_e4m3fn
    x = np.asarray(x)
    w_q = np.asarray(w_q)
    s_exp = np.asarray(s_exp)
    bias = np.asarray(bias, dtype=np.float32)
    assert x.shape == (B, T, IN) and w_q.shape == (OUT, IN)

    # Fold the power-of-two per-output-channel scale into the ternary
    # weights: values are +-2^s or 0 with s in [-8, 0], exact in fp8e4m3.
    scale = np.exp2(s_exp.astype(np.float32))
    w_scaled = w_q.astype(np.float32) * scale[:, None]  # [OUT, IN]

    # Columns sorted by s_exp descending; top RES get the residual pass.
    perm = np.argsort(-s_exp.astype(np.int64), kind="stable")
    n_top = int((s_exp >= 0).sum())
    res = RES
    if n_top > res:
        res = min(512, -(-n_top // 16) * 16)
    wp_t = np.ascontiguousarray(w_scaled[perm].T)  # [IN, OUT] permuted cols
    w_fp8 = wp_t.astype(f8)
    if not np.array_equal(w_fp8.astype(np.float32), wp_t):
        import warnings
        warnings.warn("scaled ternary weights not exact in fp8e4m3; "
                      "proceeding with rounded weights")

    # w chunk tensors: w[name][p, kp, i, o] = w[(2kp+i)*128+p, lo+o]
    w_in = {}
    for name, lo, hi, _ in CHUNKS:
        w_in[f"w_{name}"] = np.ascontiguousarray(
            w_fp8[:, lo:hi].reshape(KP, 2, P, hi - lo).transpose(2, 0, 1, 3))
    bias_p = np.ascontiguousarray(
        np.broadcast_to(bias[perm].astype(np.float16), (P, OUT)))

    # x8 = e4m3(x), xlo = e4m3(x - x8), t-major k-pair tiles
    xf = x.astype(np.float32)
    x8_full = xf.astype(f8)
    xlo_full = (xf - x8_full.astype(np.float32)).astype(f8)

    def pack_x(a):  # [T, IN] -> [TT, P, KP, 2, P]
        return a.reshape(TT, P, KP, 2, P).transpose(0, 4, 2, 3, 1)

    nc = _CACHE.get(("nc", res))
    if nc is None:
        nc = _CACHE[("nc", res)] = _build(res)

    in_maps = []
    for b in range(B):
        x8t = pack_x(x8_full[b])
        xlot = pack_x(xlo_full[b])
        m = {"x8lo": np.ascontiguousarray(x8t[:4]),
             "x8hi": np.ascontiguousarray(x8t[4:].transpose(1, 0, 2, 3, 4)),
             "xlo": np.ascontiguousarray(xlot.transpose(1, 0, 2, 3, 4)),
             "bias": bias_p}
        m.update(w_in)
        in_maps.append(m)

    trace = bool(int(os.environ.get("BITLIN_TRACE", "0")))
    if trace:
        _install_prof_shim()
    res_run = run_bass_kernel_spmd(nc, in_maps, list(range(NCORES)),
                                   trace=trace)
    last_exec_time_ns = res_run.exec_time_ns

    out = np.empty((B, T, OUT), dtype=np.float32)
    inv = np.empty_like(perm)
    inv[perm] = np.arange(OUT)
    for b in range(B):
        yb = np.empty((T, OUT), dtype=np.float16)
        for name, lo, hi, _ in CHUNKS:
            yb[:, lo:hi] = res_run.results[b][f"y_{name}"]
        out[b] = yb.astype(np.float32)[:, inv]
    return out


# revision 27
# speedup vs baseline: 1.0076x; 1.0076x over previous
"""BitLinear (ternary weight) inference kernel for Trainium2, 8-core SPMD.

Full-input contract: kernel(**inputs) takes the complete tensors and returns
the complete output. The batch dim (B=8) is sharded 1:1 onto the 8
NeuronCores; each core computes y[b] = x[b] @ (w_q * 2^s_exp)^T + bias as a
2048^3 matmul.

All-fp8 DoubleRow scheme. The accuracy gate is max|err|/absmax, and both
error and signal in column o scale with 2^s_exp[o]:
  - Output columns are permuted by s_exp descending. ALL columns run
    x8 (e4m3) x w8 (e4m3) DoubleRow matmuls (K=256/instr, 1 col/cycle =
    2x bf16 rate, the fp8 ceiling on trn2 HW).
  - The top RES=256 columns (covering all s_exp=0) get a second
    DoubleRow pass with xlo8 = e4m3(x - x8) into the same PSUM bank
    (two-term fp8, ~2^-8 relative x error). Remaining columns carry
    error scaled by 2^s <= 1/2. Measured: rel 1.15e-2 vs 2e-2 gate.
  - Weights +-2^s / 0 are EXACT in fp8e4m3.
  - PE streaming floor: 16 tiles x 8 kp x (2048 + 256) cols = 122.9us.

Schedule (driven by perfetto traces of v1/v2):
  - Framework preamble runs to ~7.2us (host event + engine init; fixed).
    Warm-up DR matmuls on an *unwritten* tile start right at preamble
    end (no memset dependency) and ride the HAM clock ramp.
  - Work = (tile, column-chunk) groups of 8 (16 with residual) DR
    matmuls into one PSUM bank; epilogue on vector adds bias -> fp16
    slice; one contiguous per-chunk store per group.
  - Chunks c1a/c1b load on sync, c2a/c2b/c3/c0 on scalar, so two
    256-col sweeps are ready by ~12us from both rings; x8 tiles + bias
    ride gpsimd SWDGE (first tiles individually for fast start, rest in
    bulk blocks to amortize the ~0.7us/issue engine cost); xlo bulks
    follow on sync (needed only by the final c0 sweep).
  - A build-time greedy orders groups against a calibrated arrival
    model (ring rates, issue costs, HAM ramp) for zero PE gaps.
  - y is stored per-chunk ([T, W] tensors) so each [128, W] store is
    one fully contiguous 64-128KB burst; host reassembles.
"""
import os

import ml_dtypes
import numpy as np

B, T, IN, OUT = 8, 2048, 2048, 2048
P = 128
NCORES = 8
KP = IN // (2 * P)  # 8 k-pairs (DoubleRow K=256 per instruction)
TT = T // P         # 16 row tiles
RES = 256           # columns (after perm) that get the xlo residual pass
NWARM = 16

# Column chunks: (name, col_lo, col_hi, residual). Ring assignment below.
CHUNKS = [
    ("c1a", 512, 768, False),
    ("c1b", 768, 1024, False),
    ("c2a", 1024, 1280, False),
    ("c2b", 1280, 1536, False),
    ("c3", 1536, 2048, False),
    ("c0", 0, 512, True),
]
SYNC_W = ("c1a", "c1b")          # weight chunks on the sync ring
SCALAR_W = ("c2a", "c2b", "c3", "c0")

# x8 tile DMA blocks on gpsimd: (start, end) tile ranges; bias rides
# after block 2 (see _build). First tile split in halves.
X8_BLOCKS = [(0, 1), (1, 2), (2, 3), (3, 4), (4, 8), (8, 12), (12, 16)]

# Arrival model (us, MB/us) calibrated on v2-v4 perfetto traces. The
# whole chip (PE, engines, DMA) starts at ~half clock and HAM ramps it
# only under sustained PE activity, so ring rates are piecewise: slow
# until the warmup chain has fed HAM (~RAMP_END), faster after.
T_PRE = 7.6      # engines free after framework preamble
T_ISSUE = 0.7    # per dma_start instruction on the issuing engine
BW_HW = (0.06, 0.20)   # HWDGE ring MB/us (pre-ramp, post-ramp)
BW_GP = (0.045, 0.15)  # gpsimd SWDGE ring
RAMP_END = 14.0  # clocks at ~half before this
PE_NS_PER_COL = 1 / 2.4e3

last_exec_time_ns = None
_CACHE = {}


def _install_prof_shim():
    """Make antenv.axon_hooks importable so trace=True works under axon."""
    import sys
    import types

    if "antenv.axon_hooks" in sys.modules:
        return
    try:
        from trn_agent_boot.trn_boot import _ntff_profile_via_ctypes
    except ImportError:
        return
    hook = _ntff_profile_via_ctypes("/opt/axon/libaxon_pjrt.so")
    mod = types.ModuleType("antenv.axon_hooks")
    mod.get_axon_ntff_profile_hook = lambda: hook
    mod.set_axon_ntff_profile_hook = lambda h: None
    sys.modules["antenv.axon_hooks"] = mod


def _ring(sizes_mb, bw):
    """Completion times for sequential ring transfers with issue costs
    and a piecewise (pre-ramp, post-ramp) bandwidth."""
    lo, hi = bw
    out = []
    t = 0.0
    for i, s in enumerate(sizes_mb):
        t = max(t, T_PRE + T_ISSUE * (i + 1))
        rem = s
        while rem > 1e-9:
            r = lo if t < RAMP_END else hi
            step = rem / r
            if t < RAMP_END:
                step = min(step, RAMP_END - t + 1e-9)
            moved = step * r
            rem -= moved
            t += step
        out.append(t)
    return out


def _arrivals():
    # sync: c1a, x8[0] halves, x8[1], c1b, then x8 pairs 2-3, 6-7, 10-11,
    # 14-15
    sy = _ring([0.25, 0.125, 0.125, 0.25, 0.25, 0.5, 0.5, 0.5, 0.5], BW_HW)
    # scalar: bias[512:1024], c2a, pair 4-5, c2b, pair 8-9, c3, pair 12-13,
    # c0
    sc = _ring([0.25, 0.25, 0.5, 0.25, 0.5, 0.5, 0.5, 0.5], BW_HW)
    arr_x8 = {0: sy[2], 1: sy[3], 2: sy[5], 3: sy[5],
              6: sy[6], 7: sy[6], 10: sy[7], 11: sy[7],
              14: sy[8], 15: sy[8],
              4: sc[2], 5: sc[2], 8: sc[4], 9: sc[4],
              12: sc[6], 13: sc[6]}
    arr_w = {"c1a": sy[0], "c1b": sy[4],
             "c2a": sc[1], "c2b": sc[3], "c3": sc[5], "c0": sc[7]}

    # gpsimd: bias slices, xlo in two bulks (needed only by c0 sweep)
    gp = _ring([0.25, 0.25, 2.0, 2.0], BW_GP)
    arr_xlo = {tt: gp[2] if tt < 8 else gp[3] for tt in range(TT)}
    arr_bias = sc[0]
    return arr_x8, arr_w, arr_xlo, arr_bias


def _schedule():
    """Greedy (chunk, tile) order from the arrival model."""
    arr_x8, arr_w, arr_xlo, _ = _arrivals()
    groups = []
    for ci, (name, lo, hi, res) in enumerate(CHUNKS):
        for tt in range(TT):
            ready = max(arr_w[name], arr_x8[tt])
            if res:
                ready = max(ready, arr_xlo[tt])
            cols = (hi - lo) + (RES if res else 0)
            groups.append([ready, ci, tt, cols])

    order = []
    t = T_PRE + 0.35 * NWARM  # warmups at ramp clock
    pend = groups
    while pend:
        ready = [g for g in pend if g[0] <= t]
        if not ready:
            t = min(g[0] for g in pend)
            ready = [g for g in pend if g[0] <= t]
        g = min(ready, key=lambda x: (x[1], x[2]))
        pend.remove(g)
        order.append((g[1], g[2]))
        dur = g[3] * 8 * PE_NS_PER_COL
        if t < RAMP_END:
            dur *= 2
        t += dur
    return order, t


def _build(res):
    import concourse.bacc as bacc
    import concourse.mybir as mybir
    from concourse.tile import TileContext

    DR = mybir.MatmulPerfMode.DoubleRow

    nc = bacc.Bacc()
    # t-major fp8 x tiles: x8lo[tt, p, kp, i, m] = x[tt*128+m, (2kp+i)*128+p]
    # for the first 2 tiles (tile-major, one DMA each for fast start);
    # the remaining 14 tiles are partition-major (x8hi) so 2-tile pairs
    # load as single 4KB-line DMAs. xlo is partition-major throughout.
    x8lo = nc.dram_tensor("x8lo", (2, P, KP, 2, P), mybir.dt.float8e4,
                          kind="ExternalInput")
    x8hi = nc.dram_tensor("x8hi", (P, TT - 2, KP, 2, P), mybir.dt.float8e4,
                          kind="ExternalInput")
    xlo = nc.dram_tensor("xlo", (P, TT, KP, 2, P), mybir.dt.float8e4,
                         kind="ExternalInput")
    wd = {}
    yd = {}
    for name, lo, hi, _ in CHUNKS:
        wd[name] = nc.dram_tensor(f"w_{name}", (P, KP, 2, hi - lo),
                                  mybir.dt.float8e4, kind="ExternalInput")
        yd[name] = nc.dram_tensor(f"y_{name}", (T, hi - lo),
                                  mybir.dt.float16, kind="ExternalOutput")
    bias = nc.dram_tensor("bias", (P, OUT), mybir.dt.float16,
                          kind="ExternalInput")

    order, _ = _schedule()

    with TileContext(nc) as tc:
        with tc.tile_pool(name="xp", bufs=1) as xp, \
             tc.tile_pool(name="wp", bufs=1) as wp, \
             tc.tile_pool(name="bp", bufs=1) as bp, \
             tc.tile_pool(name="opa", bufs=32) as opa, \
             tc.tile_pool(name="opb", bufs=20) as opb, \
             tc.tile_pool(name="pp", bufs=8, space="PSUM") as pp:

            # HAM pre-warm: small memset on gpsimd (its first instruction,
            # ~0.3us), then DR matmuls ride the clock ramp while loads land.
            warm_sb = bp.tile([P, 2, 256], mybir.dt.float8e4, tag="warm")
            nc.gpsimd.memset(warm_sb, 0.0)
            warm_ps = pp.tile([P, 256], mybir.dt.float32, tag="ps",
                              name="warmps")
            for i in range(NWARM):
                nc.tensor.matmul(warm_ps, warm_sb[:, :, 0:128], warm_sb,
                                 start=(i == 0), stop=(i == NWARM - 1),
                                 perf_mode=DR)

            # --- input loads ---
            x8_sb = [xp.tile([P, KP, 2, P], mybir.dt.float8e4,
                             tag=f"x8_{tt}", name=f"x8_{tt}")
                     for tt in range(2)]
            x8hi_sb = xp.tile([P, TT - 2, KP, 2, P], mybir.dt.float8e4,
                              tag="x8hi")
            xlo_sb = xp.tile([P, TT, KP, 2, P], mybir.dt.float8e4,
                             tag="xlo")
            bias_sb = bp.tile([P, OUT], mybir.dt.float16, tag="bias")

            w_sb = {}
            for name, lo, hi, _ in CHUNKS:
                w_sb[name] = wp.tile([P, KP, 2, hi - lo], mybir.dt.float8e4,
                                     tag=f"w_{name}", name=f"w_{name}")

            # x8 pairs j -> tiles (j+2, j+3) in x8hi coords
            def pair(j):
                nc_e = nc.sync if j in (0, 4, 8, 12) else nc.scalar
                nc_e.dma_start(x8hi_sb[:, j:j + 2], x8hi[:, j:j + 2])

            # sync HWDGE: c1a, x8[0] halves, x8[1], c1b, pairs 0,4,8,12
            nc.sync.dma_start(w_sb["c1a"], wd["c1a"][:, :, :, :])
            for q in range(0, KP, 4):
                nc.sync.dma_start(x8_sb[0][:, q:q + 4], x8lo[0, :, q:q + 4])
            nc.sync.dma_start(x8_sb[1], x8lo[1])
            nc.sync.dma_start(w_sb["c1b"], wd["c1b"][:, :, :, :])
            for j in (0, 4, 8, 12):
                pair(j)

            # scalar HWDGE: bias slice, c2a, pair, c2b, pair, c3, pair, c0
            nc.scalar.dma_start(bias_sb[:, 512:1024], bias[:, 512:1024])
            nc.scalar.dma_start(w_sb["c2a"], wd["c2a"][:, :, :, :])
            pair(2)
            nc.scalar.dma_start(w_sb["c2b"], wd["c2b"][:, :, :, :])
            pair(6)
            nc.scalar.dma_start(w_sb["c3"], wd["c3"][:, :, :, :])
            pair(10)
            nc.scalar.dma_start(w_sb["c0"], wd["c0"][:, :, :, :])

            # gpsimd SWDGE (slowest early): bias tail slices, xlo bulks
            nc.gpsimd.dma_start(bias_sb[:, 1024:1536], bias[:, 1024:1536])
            nc.gpsimd.dma_start(bias_sb[:, 1536:2048], bias[:, 1536:2048])
            nc.gpsimd.dma_start(bias_sb[:, 0:512], bias[:, 0:512])
            for j in range(0, TT, 8):
                nc.gpsimd.dma_start(xlo_sb[:, j:j + 8], xlo[:, j:j + 8])

            def x8_ap(tt, kp):
                if tt < 2:
                    return x8_sb[tt][:, kp, :, :]
                return x8hi_sb[:, tt - 2, kp, :, :]

            # --- compute groups ---
            def group(gi, ci, tt):
                name, lo, hi, has_res = CHUNKS[ci]
                w = hi - lo
                wt = w_sb[name]
                ps = pp.tile([P, w], mybir.dt.float32, tag="ps",
                             name=f"ps_{name}_{tt}")
                for kp in range(KP):
                    nc.tensor.matmul(ps, x8_ap(tt, kp),
                                     wt[:, kp, :, :],
                                     start=(kp == 0),
                                     stop=(kp == KP - 1 and not has_res),
                                     perf_mode=DR)
                if has_res:
                    for kp in range(KP):
                        nc.tensor.matmul(ps[:, :res],
                                         xlo_sb[:, tt, kp, :, :],
                                         wt[:, kp, :, 0:res],
                                         start=False, stop=(kp == KP - 1),
                                         perf_mode=DR)
                pool = opa if w == 256 else opb
                ot = pool.tile([P, w], mybir.dt.float16, tag="out",
                               name=f"ot_{name}_{tt}")
                nc.vector.tensor_add(ot, ps, bias_sb[:, lo:hi])
                eng = nc.scalar if gi % 2 == 0 else nc.sync
                eng.dma_start(yd[name][tt * P:(tt + 1) * P, :], ot)

            for gi, (ci, tt) in enumerate(order):
                group(gi, ci, tt)

    nc.compile()
    return nc


def kernel(x, w_q, s_exp, bias):
    global last_exec_time_ns
    from concourse.bass_utils import run_bass_kernel_spmd

    f8 = ml_dtypes.float8_e4m3fn
    x = np.asarray(x)
    w_q = np.asarray(w_q)
    s_exp = np.asarray(s_exp)
    bias = np.asarray(bias, dtype=np.float32)
    assert x.shape == (B, T, IN) and w_q.shape == (OUT, IN)

    # Fold the power-of-two per-output-channel scale into the ternary
    # weights: values are +-2^s or 0 with s in [-8, 0], exact in fp8e4m3.
    scale = np.exp2(s_exp.astype(np.float32))
    w_scaled = w_q.astype(np.float32) * scale[:, None]  # [OUT, IN]

    # Columns sorted by s_exp descending; top RES get the residual pass.
    perm = np.argsort(-s_exp.astype(np.int64), kind="stable")
    n_top = int((s_exp >= 0).sum())
    res = RES
    if n_top > res:
        res = min(512, -(-n_top // 16) * 16)
    wp_t = np.ascontiguousarray(w_scaled[perm].T)  # [IN, OUT] permuted cols
    w_fp8 = wp_t.astype(f8)
    if not np.array_equal(w_fp8.astype(np.float32), wp_t):
        import warnings
        warnings.warn("scaled ternary weights not exact in fp8e4m3; "
                      "proceeding with rounded weights")

    # w chunk tensors: w[name][p, kp, i, o] = w[(2kp+i)*128+p, lo+o]
    w_in = {}
    for name, lo, hi, _ in CHUNKS:
        w_in[f"w_{name}"] = np.ascontiguousarray(
            w_fp8[:, lo:hi].reshape(KP, 2, P, hi - lo).transpose(2, 0, 1, 3))
    bias_p = np.ascontiguousarray(
        np.broadcast_to(bias[perm].astype(np.float16), (P, OUT)))

    # x8 = e4m3(x), xlo = e4m3(x - x8), t-major k-pair tiles
    xf = x.astype(np.float32)
    x8_full = xf.astype(f8)
    xlo_full = (xf - x8_full.astype(np.float32)).astype(f8)

    def pack_x(a):  # [T, IN] -> [TT, P, KP, 2, P]
        return a.reshape(TT, P, KP, 2, P).transpose(0, 4, 2, 3, 1)

    nc = _CACHE.get(("nc", res))
    if nc is None:
        nc = _CACHE[("nc", res)] = _build(res)

    in_maps = []
    for b in range(B):
        x8t = pack_x(x8_full[b])
        xlot = pack_x(xlo_full[b])
        m = {"x8lo": np.ascontiguousarray(x8t[:2]),
             "x8hi": np.ascontiguousarray(x8t[2:].transpose(1, 0, 2, 3, 4)),
             "xlo": np.ascontiguousarray(xlot.transpose(1, 0, 2, 3, 4)),
             "bias": bias_p}
        m.update(w_in)
        in_maps.append(m)

    trace = bool(int(os.environ.get("BITLIN_TRACE", "0")))
    if trace:
        _install_prof_shim()
    res_run = run_bass_kernel_spmd(nc, in_maps, list(range(NCORES)),
                                   trace=trace)
    last_exec_time_ns = res_run.exec_time_ns

    out = np.empty((B, T, OUT), dtype=np.float32)
    inv = np.empty_like(perm)
    inv[perm] = np.arange(OUT)
    for b in range(B):
        yb = np.empty((T, OUT), dtype=np.float16)
        for name, lo, hi, _ in CHUNKS:
            yb[:, lo:hi] = res_run.results[b][f"y_{name}"]
        out[b] = yb.astype(np.float32)[:, inv]
    return out
